# revision 8
# baseline (speedup 1.0000x reference)
"""DeepseekV2 MoE layer on 8 Trainium2 NeuronCores (expert-parallel).

Strategy (per core m, local experts {2m, 2m+1}):
  - Router on-device from the full fp32 x (f32r matmul: fp32 data at bf16
    matmul rate for moving-dim >= 256). Gate weight columns permuted
    host-side so the core's local experts are score columns 0 and 1.
  - Top-2 via DVE max8 + is_equal masks; per-chunk masks/weights collected
    into [128, NCH] tiles, softmax denominators batched.
  - Dispatch: gpsimd sparse_gather compaction of (token_id+1)*mask - 1
    gives the slot->token list; pad slots marked token=T via num_found.
  - Payload: row-wise indirect DMA gather (token rows of bf16 x, 2KB
    contiguous descriptors), then PE transposes into [h%128, k, slot]
    matmul layout. Expert SwiGLU MLP in bf16 (fp32 PSUM); top-k weight
    folded into the PSUM->SBUF ACT copy; compact [slot, H] bf16 rows +
    slot->token ids written to DRAM; host scatter-adds.
  - Shared expert intermediate-sharded (ISS=128 per core) in f32r off the
    resident fp32 x; dense [T, H] bf16 partial written per core; host sums.
"""

import numpy as np

B, S, H = 2, 1024, 1024
E, I = 16, 512
TOP_K = 2
N_SHARED = 2
IS = I * N_SHARED
T = B * S
N_CORES = 8
EL = E // N_CORES          # local experts per core
TS = T // N_CORES          # shared-expert token slice per core
CAP = 384                  # per-expert token capacity (max actual load 286)
NCH = T // 128             # 16 token chunks
KH = H // 128              # 8 contraction chunks over H
NSC = CAP // 128           # slot chunks
IC = I // 128              # routed intermediate chunks
ISC = IS // 128            # shared intermediate chunks
ISS = IS // N_CORES        # shared intermediate slice per core

_cache = {}


def _build():
    import concourse.bass as bass
    import concourse.mybir as mybir
    import concourse.tile as tile
    from concourse import bacc
    from concourse.masks import make_identity

    f32 = mybir.dt.float32
    f32r = mybir.dt.float32r
    bf16 = mybir.dt.bfloat16
    i32 = mybir.dt.int32
    u32 = mybir.dt.uint32
    Alu = mybir.AluOpType
    Act = mybir.ActivationFunctionType

    nc = bacc.Bacc("TRN2", target_bir_lowering=False, debug=False)

    xT_d = nc.dram_tensor("xT", [H, T], f32r, kind="ExternalInput")
    x16_d = nc.dram_tensor("x16", [T, H], bf16, kind="ExternalInput")
    gwT_d = nc.dram_tensor("gwT", [H, E], f32r, kind="ExternalInput")
    wg_d = nc.dram_tensor("wg", [EL, H, I], bf16, kind="ExternalInput")
    wu_d = nc.dram_tensor("wu", [EL, H, I], bf16, kind="ExternalInput")
    wd_d = nc.dram_tensor("wd", [EL, I, H], bf16, kind="ExternalInput")
    wsg_d = nc.dram_tensor("wsg", [H, ISS], f32r, kind="ExternalInput")
    wsu_d = nc.dram_tensor("wsu", [H, ISS], f32r, kind="ExternalInput")
    wsd_d = nc.dram_tensor("wsd", [ISS, H], f32r, kind="ExternalInput")
    shared_d = nc.dram_tensor("shared", [T, H], bf16, kind="ExternalOutput")
    routed_d = nc.dram_tensor("routed", [EL * CAP, H], bf16,
                              kind="ExternalOutput")
    tos_d = nc.dram_tensor("tos", [EL, 128, NSC], i32, kind="ExternalOutput")
    nfd_d = nc.dram_tensor("nfd", [EL, 1], f32, kind="Internal")

    with tile.TileContext(nc) as tc:
        with (
            tc.tile_pool(name="res", bufs=1) as res,
            tc.tile_pool(name="ps_lg", bufs=2, space="PSUM") as ps_lg,
            tc.tile_pool(name="ps_tr", bufs=1, space="PSUM") as ps_tr,
            tc.tile_pool(name="ps_t16", bufs=2, space="PSUM") as ps_t16,
            tc.tile_pool(name="ps_mm", bufs=3, space="PSUM") as ps_mm,
        ):
            # ---------------- resident loads (issue order = arrival order) --
            gwt = res.tile([128, KH, E], f32r)
            nc.sync.dma_start(gwt[:], gwT_d.rearrange("(k p) e -> p k e", p=128))
            wk_cm = tc.tile_pool(name="wk", bufs=2)
            wk = wk_cm.__enter__()
            xtp_cm = tc.tile_pool(name="xtp", bufs=1)
            xtp = xtp_cm.__enter__()
            xt = xtp.tile([128, KH, T], f32r)
            for q in range(4):
                sl = slice(q * 512, (q + 1) * 512)
                nc.sync.dma_start(
                    xt[:, :, sl],
                    xT_d[:, sl].rearrange("(k p) t -> p k t", p=128))
            wsg = res.tile([128, KH, ISS], f32r)
            nc.sync.dma_start(wsg[:], wsg_d.rearrange("(k p) i -> p k i", p=128))
            wsu = res.tile([128, KH, ISS], f32r)
            nc.sync.dma_start(wsu[:], wsu_d.rearrange("(k p) i -> p k i", p=128))
            wsd = res.tile([128, H], f32r)
            nc.sync.dma_start(wsd[:], wsd_d[:])
            wg = res.tile([128, EL * KH, I], bf16)
            nc.sync.dma_start(wg[:], wg_d.rearrange("l (k p) i -> p (l k) i", p=128))
            wu = res.tile([128, EL * KH, I], bf16)
            nc.sync.dma_start(wu[:], wu_d.rearrange("l (k p) i -> p (l k) i", p=128))
            wd = res.tile([128, EL * IC, H], bf16)
            nc.sync.dma_start(wd[:], wd_d.rearrange("l (c p) h -> p (l c) h", p=128))
            ident32 = res.tile([128, 128], f32)
            make_identity(nc, ident32[:])
            ident16 = res.tile([128, 128], bf16)
            make_identity(nc, ident16[:])

            # iota over [16, 128]: val = 128*q + f + 1
            iota1 = res.tile([16, 128], f32)
            nc.gpsimd.iota(iota1[:], pattern=[[1, 128]], base=1,
                           channel_multiplier=128,
                           allow_small_or_imprecise_dtypes=True)
            # linear slot id in [128, NSC]: p + 128*sc
            slotid = res.tile([128, NSC], f32)
            nc.gpsimd.iota(slotid[:], pattern=[[128, NSC]], base=0,
                           channel_multiplier=1,
                           allow_small_or_imprecise_dtypes=True)

            # ---------------- router ----------------
            lgT = res.tile([16, T], f32)
            e_sb = res.tile([128, NCH, E], f32)      # exp(logits)
            ssum = res.tile([128, NCH], f32)         # sum / later 1/sum
            Mg = [res.tile([128, NCH], f32, name=f"Mg{l}") for l in range(EL)]
            Wt = [res.tile([128, NCH], f32, name=f"Wt{l}") for l in range(EL)]
            aw = [res.tile([128, NCH], f32, name=f"aw{l}") for l in range(EL)]

            for q in range(4):
                sl = slice(q * 512, (q + 1) * 512)
                lg = ps_lg.tile([16, 512], f32, tag="lg")
                for k in range(KH):
                    nc.tensor.matmul(lg[:], lhsT=gwt[:, k, :],
                                     rhs=xt[:, k, sl],
                                     start=(k == 0), stop=(k == KH - 1))
                nc.vector.tensor_copy(lgT[:, sl], lg[:])
                for c in range(q * 4, q * 4 + 4):
                    lg2 = ps_tr.tile([128, E], f32, tag="tr")
                    nc.tensor.transpose(lg2[:], lgT[:, c * 128:(c + 1) * 128],
                                        ident32[:16, :16])
                    ech = e_sb[:, c, :]
                    nc.scalar.activation(ech, lg2[:], Act.Exp)
                    nc.vector.reduce_sum(ssum[:, c:c + 1], ech,
                                         axis=mybir.AxisListType.X)
                    mx8 = wk.tile([128, 8], f32, tag="mx8")
                    nc.vector.max(mx8[:], ech)
                    mk1 = wk.tile([128, EL], f32, tag="mk1")
                    mk2 = wk.tile([128, EL], f32, tag="mk2")
                    e01 = e_sb[:, c, 0:EL]
                    nc.vector.tensor_scalar(mk1[:], e01, mx8[:, 0:1], None,
                                            op0=Alu.is_equal)
                    nc.vector.tensor_scalar(mk2[:], e01, mx8[:, 1:2], None,
                                            op0=Alu.is_equal)
                    for l in range(EL):
                        nc.vector.tensor_add(Mg[l][:, c:c + 1],
                                             mk1[:, l:l + 1], mk2[:, l:l + 1])
            # batched: 1/sum, weights, compaction input (Wt + Mg - 1)
            nc.vector.reciprocal(ssum[:], ssum[:])
            for l in range(EL):
                nc.vector.tensor_tensor(Wt[l][:], e_sb[:, :, l], Mg[l][:],
                                        op=Alu.mult)
                nc.vector.tensor_tensor(Wt[l][:], Wt[l][:], ssum[:],
                                        op=Alu.mult)
                nc.vector.tensor_add(aw[l][:], Wt[l][:], Mg[l][:])
                nc.vector.tensor_scalar_add(aw[l][:], aw[l][:], -1.0)

            # ---------------- dispatch (per local expert) ----------------
            tos_all = [None] * EL
            wlin_all = [None] * EL
            xg_all = [None] * EL
            for l in range(EL):
                mt_ps = ps_tr.tile([16, 128], f32, tag="tr")
                nc.tensor.transpose(mt_ps[:], Mg[l][:], ident32[:])
                A = wk.tile([16, 128], f32, tag="A")
                nc.vector.tensor_tensor(A[:], iota1[:], mt_ps[:], op=Alu.mult)
                nc.vector.tensor_scalar_add(A[:], A[:], -1.0)
                idxf = wk.tile([16, CAP // 16], f32, tag="idxf")
                nf = wk.tile([1, 1], u32, tag="nf")
                nc.gpsimd.sparse_gather(idxf[:], A[:], num_found=nf[:])
                nc.vector.tensor_scalar_max(idxf[:], idxf[:], 0.0)
                nc.vector.tensor_scalar_min(idxf[:], idxf[:], float(T - 1))
                nff = wk.tile([1, 1], f32, tag="nff")
                nc.vector.tensor_copy(nff[:], nf[:])
                nc.sync.dma_start(nfd_d[l:l + 1, :], nff[:])
                nfrep = wk.tile([128, 1], f32, tag="nfrep")
                nc.sync.dma_start(nfrep[:],
                                  nfd_d[l:l + 1, :].to_broadcast([128, 1]))
                # rewrap [16, CAP/16] -> linear [128, NSC]
                tosl = wk.tile([128, NSC], f32, tag="tosl")
                idv = idxf[:].rearrange("q (s g) -> q g s", g=8)
                for g in range(8):
                    nc.sync.dma_start(tosl[16 * g:16 * (g + 1), :], idv[:, g, :])
                valid = wk.tile([128, NSC], f32, tag="valid")
                nc.vector.tensor_scalar(valid[:], slotid[:], nfrep[:, :1], None,
                                        op0=Alu.is_lt)
                td1 = wk.tile([128, NSC], f32, tag="td1")
                nc.vector.tensor_tensor(td1[:], tosl[:], valid[:], op=Alu.mult)
                nc.vector.tensor_scalar(valid[:], valid[:], float(-T), float(T),
                                        op0=Alu.mult, op1=Alu.add)
                nc.vector.tensor_add(td1[:], td1[:], valid[:])
                tos_i = wk.tile([128, NSC], i32, name=f"tos{l}", bufs=1)
                nc.vector.tensor_copy(tos_i[:], td1[:])
                tos_all[l] = tos_i
                nc.sync.dma_start(tos_d[l], tos_i[:])
                # per-slot combine weight, same compaction
                awt_ps = ps_tr.tile([16, 128], f32, tag="tr")
                nc.tensor.transpose(awt_ps[:], aw[l][:], ident32[:])
                awt = wk.tile([16, 128], f32, tag="awt")
                nc.vector.tensor_copy(awt[:], awt_ps[:])
                wwrap = wk.tile([16, CAP // 16], f32, tag="wwrap")
                nfw = wk.tile([1, 1], u32, tag="nfw")
                nc.gpsimd.sparse_gather(wwrap[:], awt[:], num_found=nfw[:])
                wlin = wk.tile([128, NSC], f32, name=f"wlin{l}", bufs=1)
                wwv = wwrap[:].rearrange("q (s g) -> q g s", g=8)
                for g in range(8):
                    nc.sync.dma_start(wlin[16 * g:16 * (g + 1), :], wwv[:, g, :])
                wlin_all[l] = wlin
                # payload gather: token rows of bf16 x (pad slots skipped)
                xg = wk.tile([128, NSC, H], bf16, name=f"xg{l}", bufs=1)
                for sc in range(NSC):
                    nc.gpsimd.indirect_dma_start(
                        out=xg[:, sc, :],
                        out_offset=None,
                        in_=x16_d[:],
                        in_offset=bass.IndirectOffsetOnAxis(
                            ap=tos_i[:, sc:sc + 1], axis=0),
                        bounds_check=T - 1, oob_is_err=False)
                xg_all[l] = xg

            # ---------------- shared expert (ISS=128 intermediate slice) ---
            acts_sh = res.tile([128, T], f32r)
            for q in range(4):
                sl = slice(q * 512, (q + 1) * 512)
                g_ps = ps_mm.tile([128, 512], f32, tag="mm")
                u_ps = ps_mm.tile([128, 512], f32, tag="mm")
                for k in range(KH):
                    nc.tensor.matmul(g_ps[:], lhsT=wsg[:, k, :],
                                     rhs=xt[:, k, sl],
                                     start=(k == 0), stop=(k == KH - 1))
                for k in range(KH):
                    nc.tensor.matmul(u_ps[:], lhsT=wsu[:, k, :],
                                     rhs=xt[:, k, sl],
                                     start=(k == 0), stop=(k == KH - 1))
                sgs = wk.tile([128, 512], f32, tag="sgs")
                nc.scalar.activation(sgs[:], g_ps[:], Act.Silu)
                nc.vector.tensor_tensor(acts_sh[:, sl], sgs[:], u_ps[:],
                                        op=Alu.mult)
            xtp_cm.__exit__(None, None, None)
            for c in range(NCH):
                osh = wk.tile([128, H], bf16, tag="osh")
                for h2 in range(H // 512):
                    o_ps = ps_mm.tile([128, 512], f32, tag="mm")
                    nc.tensor.matmul(
                        o_ps[:],
                        lhsT=acts_sh[:, c * 128:(c + 1) * 128],
                        rhs=wsd[:, h2 * 512:(h2 + 1) * 512],
                        start=True, stop=True)
                    dst = osh[:, h2 * 512:(h2 + 1) * 512]
                    if h2 % 2 == 0:
                        nc.scalar.activation(dst, o_ps[:], Act.Copy)
                    else:
                        nc.vector.tensor_copy(dst, o_ps[:])
                nc.sync.dma_start(shared_d[c * 128:(c + 1) * 128, :], osh[:])

            # ---------------- routed experts ----------------
            for l in range(EL):
                xg = xg_all[l]
                wlin = wlin_all[l]
                xgT = wk.tile([128, KH, CAP], bf16, name=f"xgT{l}", bufs=1)
                for sc in range(NSC):
                    for k in range(KH):
                        tr_ps = ps_t16.tile([128, 128], bf16, tag="tr16")
                        nc.tensor.transpose(
                            tr_ps[:], xg[:, sc, k * 128:(k + 1) * 128],
                            ident16[:])
                        dst = xgT[:, k, sc * 128:(sc + 1) * 128]
                        if (sc * KH + k) % 2 == 0:
                            nc.scalar.activation(dst, tr_ps[:], Act.Copy)
                        else:
                            nc.vector.tensor_copy(dst, tr_ps[:])
                act_l = wk.tile([128, IC, CAP], bf16, name=f"act{l}", bufs=1)
                for ic in range(IC):
                    g_ps = ps_mm.tile([128, CAP], f32, tag="mm")
                    u_ps = ps_mm.tile([128, CAP], f32, tag="mm")
                    for k in range(KH):
                        nc.tensor.matmul(
                            g_ps[:], lhsT=wg[:, l * KH + k, ic * 128:(ic + 1) * 128],
                            rhs=xgT[:, k, :], start=(k == 0), stop=(k == KH - 1))
                    for k in range(KH):
                        nc.tensor.matmul(
                            u_ps[:], lhsT=wu[:, l * KH + k, ic * 128:(ic + 1) * 128],
                            rhs=xgT[:, k, :], start=(k == 0), stop=(k == KH - 1))
                    gs = wk.tile([128, CAP], f32, tag="gs")
                    nc.scalar.activation(gs[:], g_ps[:], Act.Silu)
                    nc.vector.tensor_tensor(act_l[:, ic, :], gs[:], u_ps[:],
                                            op=Alu.mult)
                for sc in range(NSC):
                    ysb = wk.tile([128, H], bf16, tag="ysb")
                    for h2 in range(H // 512):
                        y_ps = ps_mm.tile([128, 512], f32, tag="mm")
                        for ic in range(IC):
                            nc.tensor.matmul(
                                y_ps[:],
                                lhsT=act_l[:, ic, sc * 128:(sc + 1) * 128],
                                rhs=wd[:, l * IC + ic, h2 * 512:(h2 + 1) * 512],
                                start=(ic == 0), stop=(ic == IC - 1))
                        nc.scalar.activation(ysb[:, h2 * 512:(h2 + 1) * 512],
                                             y_ps[:], Act.Copy,
                                             scale=wlin[:, sc:sc + 1])
                    nc.sync.dma_start(
                        routed_d[(l * NSC + sc) * 128:(l * NSC + sc + 1) * 128, :],
                        ysb[:])
            wk_cm.__exit__(None, None, None)

    nc.compile()
    return nc


def _get_nc():
    if "nc" not in _cache:
        _cache["nc"] = _build()
    return _cache["nc"]


def make_in_maps(hidden_states, gate_w, w_gate, w_up, w_down,
                 ws_gate, ws_up, ws_down):
    import ml_dtypes
    bf = ml_dtypes.bfloat16
    x = np.asarray(hidden_states, np.float32).reshape(T, H)
    xT = np.ascontiguousarray(x.T)
    x16 = x.astype(bf)
    gate_w = np.asarray(gate_w, np.float32)
    w_gate = np.asarray(w_gate, np.float32)
    w_up = np.asarray(w_up, np.float32)
    w_down = np.asarray(w_down, np.float32)
    ws_gate = np.asarray(ws_gate, np.float32)
    ws_up = np.asarray(ws_up, np.float32)
    ws_down = np.asarray(ws_down, np.float32)
    in_maps = []
    for m in range(N_CORES):
        loc = [EL * m + j for j in range(EL)]
        perm = loc + [e for e in range(E) if e not in loc]
        in_maps.append({
            "xT": xT,
            "x16": x16,
            "gwT": np.ascontiguousarray(gate_w[perm].T),
            "wg": np.ascontiguousarray(w_gate[loc]).astype(bf),
            "wu": np.ascontiguousarray(w_up[loc]).astype(bf),
            "wd": np.ascontiguousarray(w_down[loc]).astype(bf),
            "wsg": np.ascontiguousarray(ws_gate[:, ISS * m:ISS * (m + 1)]),
            "wsu": np.ascontiguousarray(ws_up[:, ISS * m:ISS * (m + 1)]),
            "wsd": np.ascontiguousarray(ws_down[ISS * m:ISS * (m + 1), :]),
        })
    return in_maps


def kernel(hidden_states, gate_w, w_gate, w_up, w_down,
           ws_gate, ws_up, ws_down, _trace=False):
    from concourse import bass_utils
    nc = _get_nc()
    in_maps = make_in_maps(hidden_states, gate_w, w_gate, w_up, w_down,
                           ws_gate, ws_up, ws_down)
    res = bass_utils.run_bass_kernel_spmd(
        nc, in_maps, core_ids=list(range(N_CORES)), trace=_trace)
    _cache["last_results"] = res
    out = np.zeros((T, H), np.float32)
    for m in range(N_CORES):
        out += np.asarray(res.results[m]["shared"]).astype(np.float32)
    for m in range(N_CORES):
        routed = np.asarray(res.results[m]["routed"]).astype(np.float32)
        tos = np.asarray(res.results[m]["tos"])
        for l in range(EL):
            ids = tos[l].T.reshape(-1)          # linear slot j = 128*sc + p
            rows = routed[l * CAP:(l + 1) * CAP]
            msk = ids < T
            out[ids[msk]] += rows[msk]
    return out.reshape(B, S, H)


# revision 9
# speedup vs baseline: 1.0498x; 1.0498x over previous
"""DeepseekV2 MoE layer on 8 Trainium2 NeuronCores (expert-parallel).

Strategy (per core m, local experts {2m, 2m+1}):
  - Router logits on-device from the full fp32 x (f32r matmul: fp32 data at
    bf16 matmul rate). Gate weight columns permuted host-side so the core's
    local experts are score columns 0 and 1. Top-2 picks via DVE max8 +
    is_equal on RAW logits (softmax is monotone, so picks are identical);
    the logits are dumped to DRAM and the exact softmax combine weights are
    applied host-side during the scatter-add.
  - Dispatch: gpsimd sparse_gather compaction of (token_id+1)*mask - 1 per
    expert; the first num_found linear slots are valid (round-robin fill),
    num_found is dumped for the host; pad slots compute garbage that the
    host drops.
  - Payload: row-wise indirect DMA gather of bf16 token rows (2KB
    descriptors), PE-transposed into [h%128, k, slot] layout. Expert SwiGLU
    MLP in bf16 (fp32 PSUM); unscaled [slot, H] bf16 rows + slot->token ids
    written to DRAM.
  - Shared expert intermediate-sharded (ISS=128 per core) in f32r off the
    resident fp32 x, interleaved with the router per 512-token quarter so
    the PE tracks the x DMA arrival; dense [T, H] bf16 partial per core.
  - Host: sums shared partials, softmax(logits) weights, scatter-adds the
    weighted routed rows.
"""

import numpy as np

B, S, H = 2, 1024, 1024
E, I = 16, 512
TOP_K = 2
N_SHARED = 2
IS = I * N_SHARED
T = B * S
N_CORES = 8
EL = E // N_CORES          # local experts per core
CAP = 384                  # per-expert token capacity (max actual load 286)
NCH = T // 128             # 16 token chunks
KH = H // 128              # 8 contraction chunks over H
NSC = CAP // 128           # slot chunks
IC = I // 128              # routed intermediate chunks
ISS = IS // N_CORES        # shared intermediate slice per core

_cache = {}


def _build():
    import concourse.bass as bass
    import concourse.mybir as mybir
    import concourse.tile as tile
    from concourse import bacc
    from concourse.masks import make_identity

    f32 = mybir.dt.float32
    f32r = mybir.dt.float32r
    bf16 = mybir.dt.bfloat16
    i32 = mybir.dt.int32
    u32 = mybir.dt.uint32
    Alu = mybir.AluOpType
    Act = mybir.ActivationFunctionType

    nc = bacc.Bacc("TRN2", target_bir_lowering=False, debug=False)

    xT_d = nc.dram_tensor("xT", [H, T], f32r, kind="ExternalInput")
    x16_d = nc.dram_tensor("x16", [T, H], bf16, kind="ExternalInput")
    gwT_d = nc.dram_tensor("gwT", [H, E], f32r, kind="ExternalInput")
    wg_d = nc.dram_tensor("wg", [EL, H, I], bf16, kind="ExternalInput")
    wu_d = nc.dram_tensor("wu", [EL, H, I], bf16, kind="ExternalInput")
    wd_d = nc.dram_tensor("wd", [EL, I, H], bf16, kind="ExternalInput")
    wsg_d = nc.dram_tensor("wsg", [H, ISS], f32r, kind="ExternalInput")
    wsu_d = nc.dram_tensor("wsu", [H, ISS], f32r, kind="ExternalInput")
    wsd_d = nc.dram_tensor("wsd", [ISS, H], f32r, kind="ExternalInput")
    shared_d = nc.dram_tensor("shared", [T, H], bf16, kind="ExternalOutput")
    routed_d = nc.dram_tensor("routed", [EL * CAP, H], bf16,
                              kind="ExternalOutput")
    tos_d = nc.dram_tensor("tos", [EL, 128, NSC], i32, kind="ExternalOutput")
    lg_d = nc.dram_tensor("lg", [16, T], f32, kind="ExternalOutput")
    nf_d = nc.dram_tensor("nf", [EL, 1], f32, kind="ExternalOutput")

    with tile.TileContext(nc) as tc:
        with (
            tc.tile_pool(name="res", bufs=1) as res,
            tc.tile_pool(name="ps_lg", bufs=1, space="PSUM") as ps_lg,
            tc.tile_pool(name="ps_sc", bufs=2, space="PSUM") as ps_sc,
            tc.tile_pool(name="ps_t16", bufs=2, space="PSUM") as ps_t16,
            tc.tile_pool(name="ps_mm", bufs=3, space="PSUM") as ps_mm,
        ):
            # ---------------- resident loads (issue order = arrival order) --
            gwt = res.tile([128, KH, E], f32r)
            nc.sync.dma_start(gwt[:], gwT_d.rearrange("(k p) e -> p k e", p=128))
            wk_cm = tc.tile_pool(name="wk", bufs=2)
            wk = wk_cm.__enter__()
            xtp_cm = tc.tile_pool(name="xtp", bufs=1)
            xtp = xtp_cm.__enter__()
            xt = xtp.tile([128, KH, T], f32r)
            for q in range(4):
                sl = slice(q * 512, (q + 1) * 512)
                nc.sync.dma_start(
                    xt[:, :, sl],
                    xT_d[:, sl].rearrange("(k p) t -> p k t", p=128))
            wsg = res.tile([128, KH, ISS], f32r)
            nc.sync.dma_start(wsg[:], wsg_d.rearrange("(k p) i -> p k i", p=128))
            wsu = res.tile([128, KH, ISS], f32r)
            nc.sync.dma_start(wsu[:], wsu_d.rearrange("(k p) i -> p k i", p=128))
            wsd = res.tile([128, H], f32r)
            nc.sync.dma_start(wsd[:], wsd_d[:])
            wg = res.tile([128, EL * KH, I], bf16)
            nc.sync.dma_start(wg[:], wg_d.rearrange("l (k p) i -> p (l k) i", p=128))
            wu = res.tile([128, EL * KH, I], bf16)
            nc.sync.dma_start(wu[:], wu_d.rearrange("l (k p) i -> p (l k) i", p=128))
            wd = res.tile([128, EL * IC, H], bf16)
            nc.sync.dma_start(wd[:], wd_d.rearrange("l (c p) h -> p (l c) h", p=128))
            ident32 = res.tile([128, 128], f32)
            make_identity(nc, ident32[:])
            ident16 = res.tile([128, 128], bf16)
            make_identity(nc, ident16[:])

            # iota over [16, 128]: val = 128*q + f + 1
            iota1 = res.tile([16, 128], f32)
            nc.gpsimd.iota(iota1[:], pattern=[[1, 128]], base=1,
                           channel_multiplier=128,
                           allow_small_or_imprecise_dtypes=True)

            # ------------- router + shared expert, per 512-token quarter ----
            lgT = res.tile([16, T], f32)
            Mg = [res.tile([128, NCH], f32, name=f"Mg{l}") for l in range(EL)]
            acts_sh = res.tile([128, T], f32r)
            for q in range(4):
                sl = slice(q * 512, (q + 1) * 512)
                lg = ps_lg.tile([16, 512], f32, tag="lg")
                for k in range(KH):
                    nc.tensor.matmul(lg[:], lhsT=gwt[:, k, :],
                                     rhs=xt[:, k, sl],
                                     start=(k == 0), stop=(k == KH - 1))
                nc.vector.tensor_copy(lgT[:, sl], lg[:])
                # shared expert gate/up for this quarter
                g_ps = ps_mm.tile([128, 512], f32, tag="mm")
                u_ps = ps_mm.tile([128, 512], f32, tag="mm")
                for k in range(KH):
                    nc.tensor.matmul(g_ps[:], lhsT=wsg[:, k, :],
                                     rhs=xt[:, k, sl],
                                     start=(k == 0), stop=(k == KH - 1))
                for k in range(KH):
                    nc.tensor.matmul(u_ps[:], lhsT=wsu[:, k, :],
                                     rhs=xt[:, k, sl],
                                     start=(k == 0), stop=(k == KH - 1))
                sgs = wk.tile([128, 512], f32, tag="sgs")
                nc.scalar.activation(sgs[:], g_ps[:], Act.Silu)
                nc.vector.tensor_tensor(acts_sh[:, sl], sgs[:], u_ps[:],
                                        op=Alu.mult)
                # top-2 membership masks on raw logits for this quarter
                for c in range(q * 4, q * 4 + 4):
                    lg2 = ps_sc.tile([128, E], f32, tag="sc")
                    nc.tensor.transpose(lg2[:], lgT[:, c * 128:(c + 1) * 128],
                                        ident32[:16, :16])
                    mx8 = wk.tile([128, 8], f32, tag="mx8")
                    nc.vector.max(mx8[:], lg2[:])
                    mk1 = wk.tile([128, EL], f32, tag="mk1")
                    mk2 = wk.tile([128, EL], f32, tag="mk2")
                    nc.vector.tensor_scalar(mk1[:], lg2[:, 0:EL], mx8[:, 0:1],
                                            None, op0=Alu.is_equal)
                    nc.vector.tensor_scalar(mk2[:], lg2[:, 0:EL], mx8[:, 1:2],
                                            None, op0=Alu.is_equal)
                    for l in range(EL):
                        nc.vector.tensor_add(Mg[l][:, c:c + 1],
                                             mk1[:, l:l + 1], mk2[:, l:l + 1])
                # shared expert down-proj for this quarter's chunks
                for c in range(q * 4, q * 4 + 4):
                    osh = wk.tile([128, H], bf16, tag="osh")
                    for h2 in range(H // 512):
                        o_ps = ps_mm.tile([128, 512], f32, tag="mm")
                        nc.tensor.matmul(
                            o_ps[:],
                            lhsT=acts_sh[:, c * 128:(c + 1) * 128],
                            rhs=wsd[:, h2 * 512:(h2 + 1) * 512],
                            start=True, stop=True)
                        dst = osh[:, h2 * 512:(h2 + 1) * 512]
                        if h2 % 2 == 0:
                            nc.scalar.activation(dst, o_ps[:], Act.Copy)
                        else:
                            nc.vector.tensor_copy(dst, o_ps[:])
                    nc.sync.dma_start(shared_d[c * 128:(c + 1) * 128, :],
                                      osh[:])
            nc.sync.dma_start(lg_d[:], lgT[:])
            xtp_cm.__exit__(None, None, None)

            # ---------------- dispatch (per local expert) ----------------
            tos_all = [None] * EL
            xg_all = [None] * EL
            for l in range(EL):
                mt_ps = ps_sc.tile([16, 128], f32, tag="sc")
                nc.tensor.transpose(mt_ps[:], Mg[l][:], ident32[:])
                A = wk.tile([16, 128], f32, tag="A")
                nc.vector.tensor_tensor(A[:], iota1[:], mt_ps[:], op=Alu.mult)
                nc.vector.tensor_scalar_add(A[:], A[:], -1.0)
                idxf = wk.tile([16, CAP // 16], f32, name=f"idxf{l}", bufs=1)
                nf = wk.tile([1, 1], u32, tag="nf")
                nc.gpsimd.memset(idxf[:], 0.0)
                nc.gpsimd.sparse_gather(idxf[:], A[:], num_found=nf[:])
                nc.vector.tensor_scalar_max(idxf[:], idxf[:], 0.0)
                nc.vector.tensor_scalar_min(idxf[:], idxf[:], float(T - 1))
                nff = wk.tile([1, 1], f32, tag="nff")
                nc.vector.tensor_copy(nff[:], nf[:])
                nc.sync.dma_start(nf_d[l:l + 1, :], nff[:])
                # rewrap [16, CAP/16] -> linear [128, NSC]; slot j = 128*sc+p
                tosl = wk.tile([128, NSC], f32, tag="tosl")
                idv = idxf[:].rearrange("q (s g) -> q g s", g=8)
                for g in range(8):
                    nc.sync.dma_start(tosl[16 * g:16 * (g + 1), :], idv[:, g, :])
                tos_i = wk.tile([128, NSC], i32, name=f"tos{l}", bufs=1)
                nc.vector.tensor_copy(tos_i[:], tosl[:])
                tos_all[l] = tos_i
                nc.sync.dma_start(tos_d[l], tos_i[:])
            for l in range(EL):
                tos_i = tos_all[l]
                xg = wk.tile([128, NSC, H], bf16, name=f"xg{l}", bufs=1)
                for sc in range(NSC):
                    nc.gpsimd.indirect_dma_start(
                        out=xg[:, sc, :],
                        out_offset=None,
                        in_=x16_d[:],
                        in_offset=bass.IndirectOffsetOnAxis(
                            ap=tos_i[:, sc:sc + 1], axis=0),
                        bounds_check=T - 1, oob_is_err=False)
                xg_all[l] = xg

            # ---------------- routed experts ----------------
            for l in range(EL):
                xg = xg_all[l]
                xgT = wk.tile([128, KH, CAP], bf16, name=f"xgT{l}", bufs=1)
                for sc in range(NSC):
                    for k in range(KH):
                        tr_ps = ps_t16.tile([128, 128], bf16, tag="tr16")
                        nc.tensor.transpose(
                            tr_ps[:], xg[:, sc, k * 128:(k + 1) * 128],
                            ident16[:])
                        dst = xgT[:, k, sc * 128:(sc + 1) * 128]
                        if (sc * KH + k) % 2 == 0:
                            nc.scalar.activation(dst, tr_ps[:], Act.Copy)
                        else:
                            nc.vector.tensor_copy(dst, tr_ps[:])
                act_l = wk.tile([128, IC, CAP], bf16, name=f"act{l}", bufs=1)
                for ic in range(IC):
                    g_ps = ps_mm.tile([128, CAP], f32, tag="mm")
                    u_ps = ps_mm.tile([128, CAP], f32, tag="mm")
                    for k in range(KH):
                        nc.tensor.matmul(
                            g_ps[:], lhsT=wg[:, l * KH + k, ic * 128:(ic + 1) * 128],
                            rhs=xgT[:, k, :], start=(k == 0), stop=(k == KH - 1))
                    for k in range(KH):
                        nc.tensor.matmul(
                            u_ps[:], lhsT=wu[:, l * KH + k, ic * 128:(ic + 1) * 128],
                            rhs=xgT[:, k, :], start=(k == 0), stop=(k == KH - 1))
                    gs = wk.tile([128, CAP], f32, tag="gs")
                    nc.scalar.activation(gs[:], g_ps[:], Act.Silu)
                    nc.vector.tensor_tensor(act_l[:, ic, :], gs[:], u_ps[:],
                                            op=Alu.mult)
                for sc in range(NSC):
                    ysb = wk.tile([128, H], bf16, tag="ysb")
                    for h2 in range(H // 512):
                        y_ps = ps_mm.tile([128, 512], f32, tag="mm")
                        for ic in range(IC):
                            nc.tensor.matmul(
                                y_ps[:],
                                lhsT=act_l[:, ic, sc * 128:(sc + 1) * 128],
                                rhs=wd[:, l * IC + ic, h2 * 512:(h2 + 1) * 512],
                                start=(ic == 0), stop=(ic == IC - 1))
                        dst = ysb[:, h2 * 512:(h2 + 1) * 512]
                        if h2 % 2 == 0:
                            nc.scalar.activation(dst, y_ps[:], Act.Copy)
                        else:
                            nc.vector.tensor_copy(dst, y_ps[:])
                    nc.sync.dma_start(
                        routed_d[(l * NSC + sc) * 128:(l * NSC + sc + 1) * 128, :],
                        ysb[:])
            wk_cm.__exit__(None, None, None)

    nc.compile()
    return nc


def _get_nc():
    if "nc" not in _cache:
        _cache["nc"] = _build()
    return _cache["nc"]


def make_in_maps(hidden_states, gate_w, w_gate, w_up, w_down,
                 ws_gate, ws_up, ws_down):
    import ml_dtypes
    bf = ml_dtypes.bfloat16
    x = np.asarray(hidden_states, np.float32).reshape(T, H)
    xT = np.ascontiguousarray(x.T)
    x16 = x.astype(bf)
    gate_w = np.asarray(gate_w, np.float32)
    w_gate = np.asarray(w_gate, np.float32)
    w_up = np.asarray(w_up, np.float32)
    w_down = np.asarray(w_down, np.float32)
    ws_gate = np.asarray(ws_gate, np.float32)
    ws_up = np.asarray(ws_up, np.float32)
    ws_down = np.asarray(ws_down, np.float32)
    in_maps = []
    for m in range(N_CORES):
        loc = [EL * m + j for j in range(EL)]
        perm = loc + [e for e in range(E) if e not in loc]
        in_maps.append({
            "xT": xT,
            "x16": x16,
            "gwT": np.ascontiguousarray(gate_w[perm].T),
            "wg": np.ascontiguousarray(w_gate[loc]).astype(bf),
            "wu": np.ascontiguousarray(w_up[loc]).astype(bf),
            "wd": np.ascontiguousarray(w_down[loc]).astype(bf),
            "wsg": np.ascontiguousarray(ws_gate[:, ISS * m:ISS * (m + 1)]),
            "wsu": np.ascontiguousarray(ws_up[:, ISS * m:ISS * (m + 1)]),
            "wsd": np.ascontiguousarray(ws_down[ISS * m:ISS * (m + 1), :]),
        })
    return in_maps


def kernel(hidden_states, gate_w, w_gate, w_up, w_down,
           ws_gate, ws_up, ws_down, _trace=False):
    from concourse import bass_utils
    nc = _get_nc()
    in_maps = make_in_maps(hidden_states, gate_w, w_gate, w_up, w_down,
                           ws_gate, ws_up, ws_down)
    res = bass_utils.run_bass_kernel_spmd(
        nc, in_maps, core_ids=list(range(N_CORES)), trace=_trace)
    _cache["last_results"] = res
    out = np.zeros((T, H), np.float32)
    for m in range(N_CORES):
        out += np.asarray(res.results[m]["shared"]).astype(np.float32)
    for m in range(N_CORES):
        routed = np.asarray(res.results[m]["routed"]).astype(np.float32)
        tos = np.asarray(res.results[m]["tos"])
        nf = np.asarray(res.results[m]["nf"]).reshape(EL)
        # softmax combine weights from the device's own (permuted) logits
        lg = np.asarray(res.results[m]["lg"], np.float64)      # [16, T]
        z = np.exp(lg - lg.max(axis=0, keepdims=True))
        w = (z / z.sum(axis=0, keepdims=True)).astype(np.float32)  # [E, T]
        for l in range(EL):
            n = int(round(float(nf[l])))
            ids = tos[l].T.reshape(-1)[:n]   # linear slot j = 128*sc + p
            rows = routed[l * CAP:l * CAP + n]
            out[ids] += rows * w[l, ids][:, None]
    return out.reshape(B, S, H)
